# revision 7
# baseline (speedup 1.0000x reference)
"""Trainium2 Bass kernel for nn_HadamardTransform: out = fwht(s * x) / 64.

Algorithm (per row v of length 4096, viewed as v[a*128 + m*32 + b], a,b in [0,32),
m in [0,4)):   y = (H32_a (x) H4_m (x) H32_b) v / 64
  - H32 factors run on the TensorEngine as block-diag(4 x H32)/8 matmuls
    contracting the partition dimension.
  - H4 = H2 (x) H2 runs as +/- PSUM accumulation pairs folded into the two
    matmul passes (sign via +/- weight matrices).
  - Between the passes, the b-index is moved into partitions with the DVE
    32x32 block transpose (which doubles as the mandatory PSUM->SBUF copy).
  - s-multiply runs on GPSIMD (never contends with DVE 1x ops for SBUF ports).
Sharding: rows 16384 split evenly across 8 cores (data parallel); s replicated.

SBUF layouts per 128-row block (rows r = blk*128 + t*32 + 4g + rho):
  IN/Z [32rho + a, t*1024 + g*128 + m*32 + b]   (m = 2mu + nu)
  P1   [32rho + a', mup*512 + g*64 + nu*32 + b]   (PSUM, per subtile t)
  T1   [32rho + b,  mup*512 + g*64 + nu*32 + a']
  P2   [32rho + b', nup*512 + mup*256 + g*32 + a'] (PSUM)
  OUT  [32rho + a', t*1024 + g*128 + (2mup+nup)*32 + b']
"""

import numpy as np

SIZE = 4096
ROWS = 16384
N_CORES = 8
ROWS_PER_CORE = ROWS // N_CORES  # 2048

BLOCK_ROWS = 128          # rows per DMA/smul block
SUB_ROWS = 32             # rows per compute subtile
G = SUB_ROWS // 4         # 8 row-groups of 4 adjacent rows per subtile
SUBS = BLOCK_ROWS // SUB_ROWS  # 4
FREE_PER_BLOCK = BLOCK_ROWS * SIZE // 128  # 4096

_BUILD_CACHE = {}


def _hadamard(n):
    h = np.array([[1.0]], dtype=np.float32)
    while h.shape[0] < n:
        h = np.block([[h, h], [h, -h]])
    return h.astype(np.float32)


def _weights():
    h32 = _hadamard(32)
    wp = np.kron(np.eye(4, dtype=np.float32), h32) / 8.0
    return np.ascontiguousarray(wp), np.ascontiguousarray(-wp)


def build_nc(rows_per_core=ROWS_PER_CORE):
    """Build the Bass program (single-core view; run SPMD on 8 cores)."""
    import concourse.bacc as bacc
    import concourse.bass as bass
    import concourse.mybir as mybir
    from concourse import tile

    f32 = mybir.dt.float32
    n_blocks = rows_per_core // BLOCK_ROWS

    nc = bacc.Bacc(None, target_bir_lowering=False)
    x_in = nc.dram_tensor("x", [rows_per_core, SIZE], f32, kind="ExternalInput")
    srep_in = nc.dram_tensor("s_rep", [128, 128], f32, kind="ExternalInput")
    y_out = nc.dram_tensor("y", [rows_per_core, SIZE], f32, kind="ExternalOutput")

    wp_np, wm_np = _weights()
    wp_dram = nc.inline_tensor(wp_np, "wp")
    wm_dram = nc.inline_tensor(wm_np, "wm")

    with tile.TileContext(nc) as tc:
        with (
            tc.tile_pool(name="const", bufs=1) as cpool,
            tc.tile_pool(name="inb", bufs=2) as inpool,
            tc.tile_pool(name="zb", bufs=2) as zpool,
            tc.tile_pool(name="t1b", bufs=3) as tpool,
            tc.tile_pool(name="outb", bufs=2) as opool,
            tc.tile_pool(name="ps1", bufs=2, space="PSUM") as ps1pool,
            tc.tile_pool(name="ps2", bufs=2, space="PSUM") as ps2pool,
        ):
            wp = cpool.tile([128, 128], f32, tag="wp")
            wm = cpool.tile([128, 128], f32, tag="wm")
            srep = cpool.tile([128, 128], f32, tag="srep")
            nc.sync.dma_start(wp[:], wp_dram[:])
            nc.sync.dma_start(wm[:], wm_dram[:])
            nc.sync.dma_start(srep[:], srep_in[:])

            for blk in range(n_blocks):
                # ---- load 128 rows ----
                src = x_in[blk * BLOCK_ROWS:(blk + 1) * BLOCK_ROWS, :].rearrange(
                    "(t g r) (a i) -> (r a) t g i", t=SUBS, g=G, r=4, a=32, i=128
                )
                tin = inpool.tile([128, FREE_PER_BLOCK], f32, tag="in")
                dst = tin[:].rearrange("p (t g i) -> p t g i", t=SUBS, g=G)
                nc.sync.dma_start(dst, src)

                # ---- s multiply (GPSIMD) ----
                z = zpool.tile([128, FREE_PER_BLOCK], f32, tag="z")
                z3 = z[:].rearrange("p (c i) -> p c i", i=128)
                in3 = tin[:].rearrange("p (c i) -> p c i", i=128)
                s3 = srep[:].unsqueeze(1).broadcast_to(
                    (128, FREE_PER_BLOCK // 128, 128)
                )
                nc.gpsimd.tensor_mul(z3, in3, s3)

                out = opool.tile([128, FREE_PER_BLOCK], f32, tag="out")
                # z free viewed as (t, g, m, b32):
                zv = z[:].rearrange(
                    "p (t g m i) -> p t m g i", t=SUBS, g=G, m=4, i=32
                )
                h4 = _hadamard(4)

                for t in range(SUBS):
                    # ---- MM1: contract a (H32), accumulate H4 over m ----
                    # P1 free = m'*256 + g*32 + b  (m'-slices; 2 banks)
                    p1 = ps1pool.tile([128, 1024], f32, tag="p1")
                    for mp in range(4):
                        for m in range(4):
                            w = wp if h4[mp, m] > 0 else wm
                            nc.tensor.matmul(
                                p1[:, mp * 256:(mp + 1) * 256],
                                w[:],
                                zv[:, t, m],
                                start=(m == 0),
                                stop=(m == 3),
                            )
                    # ---- vT1: b -> partitions (PSUM -> SBUF, positional) ----
                    t1 = tpool.tile([128, 1024], f32, tag="t1")
                    nc.vector.transpose(t1[:], p1[:])

                    # ---- MM2: contract b (H32), no accumulation ----
                    # out P2 free = g*128 + m'*32 + a' (contiguous), via
                    # reordered rhs reads of T1 (free = m'*256 + g*32 + a').
                    p2 = ps2pool.tile([128, 1024], f32, tag="p2")
                    t1v = t1[:].rearrange(
                        "p (mp gh gl i) -> p gh gl mp i",
                        mp=4, gh=2, gl=G // 2, i=32,
                    )
                    for gh in range(2):
                        nc.tensor.matmul(
                            p2[:, gh * 512:(gh + 1) * 512],
                            wp[:],
                            t1v[:, gh],
                            start=True,
                            stop=True,
                        )

                    # ---- vT2: a' -> partitions (contiguous, positional) ----
                    nc.vector.transpose(
                        out[:, t * 1024:(t + 1) * 1024], p2[:]
                    )

                # ---- store 128 rows ----
                dst_d = y_out[blk * BLOCK_ROWS:(blk + 1) * BLOCK_ROWS, :].rearrange(
                    "(t g r) (a i) -> (r a) t g i", t=SUBS, g=G, r=4, a=32, i=128
                )
                src_s = out[:].rearrange("p (t g i) -> p t g i", t=SUBS, g=G)
                nc.scalar.dma_start(dst_d, src_s)

    nc.compile()
    return nc


def _get_nc(rows_per_core=ROWS_PER_CORE):
    key = rows_per_core
    if key not in _BUILD_CACHE:
        _BUILD_CACHE[key] = build_nc(rows_per_core)
    return _BUILD_CACHE[key]


def make_s_rep(s):
    # s_rep[32*rho + a, m*32 + b] = s[a*128 + m*32 + b]
    return np.ascontiguousarray(np.tile(s.reshape(32, 128), (4, 1)).astype(np.float32))


def run(x, s, **kwargs):
    from concourse.bass_utils import run_bass_kernel_spmd

    x = np.ascontiguousarray(x, dtype=np.float32)
    s = np.ascontiguousarray(s, dtype=np.float32)
    nc = _get_nc()
    s_rep = make_s_rep(s)
    shards = x.reshape(N_CORES, ROWS_PER_CORE, SIZE)
    in_maps = [{"x": shards[i], "s_rep": s_rep} for i in range(N_CORES)]
    res = run_bass_kernel_spmd(nc, in_maps, list(range(N_CORES)), **kwargs)
    y = np.concatenate([r["y"] for r in res.results], axis=0)
    return y, res


def kernel(x, s):
    return run(x, s)[0]
